# revision 8
# baseline (speedup 1.0000x reference)
"""Trainium2 Bass kernel for nn_ActSeries: 20 layers of per-channel range-norm +
quadratic polynomial, x [32,32,256,256] f32.

Strategy
--------
Shard the 32 *channels* across the 8 cores (4 channels/core). The per-layer
min/max reduction is over (B,H,W) per channel, so with channel sharding every
reduction is core-local: zero collectives. One channel is 32*256*256 floats
= 8 MB, which fits in SBUF, so each channel is loaded once, run through all
20 layers on-chip, and stored once (2 HBM passes total instead of 40+).

Math (Form B, validated vs the reference to ~1.2e-6 rel):
The stored tensor W relates to the true h by h = a2_prev * W + const (the
range-norm is invariant to the affine map, tracked exactly in scalar math:
rc = a2_prev, with rc_0 = 1). Per layer, with exact stats mn,mx of W:
  g = (rc>=0); m* = g*mn + (1-g)*mx        # the stored value mapping to min h
  Delta = (mx-mn)*|rc|                     # true h range
  s = 1/(Delta+eps); q = s*rc; A = Delta*s
  u = q*W + (-q*m*)                        # == xh, in [0,A]; ONE ACT pass
  dhat = a1/a2                             # a2 sign-clamped to |a2|>=1e-27
  W' = (u + dhat) * u                      # ONE DVE scalar_tensor_tensor
       (true h' = a2*W' + a0; W' is convex in u since its leading coeff
        1/q^2 > 0, so max over data = max(0, A*(A+dhat)) from the interval
        endpoints, and only the MIN needs a data scan)
  mn' = scan-min(W')  -- fused into the copy-back tensor_scalar (accum_out)
  mx' = max(0, A*(A+dhat))
  rc' = a2
Last layer: y = a2*W' + a0 in one ACT pass (scale/bias APs).

Engine split per layer-channel: ACT computes u; DVE does the fused
(u+dhat)*u and the copy-back-with-min-accum; GPSIMD does the cross-partition
all-reduce of per-partition minima (negate + allreduce-max + negate).
"""

import os
import sys

import numpy as np

B, C, H, Wd = 32, 32, 256, 256
N_LAYERS = 20
EPS = 1e-5
N_CORES = 8
CH_PER_CORE = C // N_CORES  # 4
F_FULL = B * H * Wd // 128  # 16384 free-dim elements per partition


def _import_concourse():
    try:
        import concourse  # noqa: F401
    except ImportError:
        for p in ("/opt/trn_rl_repo", os.path.expanduser("~/.axon_site/_ro/trn_rl_repo")):
            if os.path.isdir(p) and p not in sys.path:
                sys.path.insert(0, p)
        import concourse  # noqa: F401


def build_nc(F=F_FULL, CW=2048, n_ch=CH_PER_CORE, enable_asserts=False):
    """Build the (single, SPMD) Bass program. Returns the compiled nc."""
    _import_concourse()
    import concourse.bacc as bacc
    import concourse.tile as tile
    from concourse import bass_isa, mybir

    f32 = mybir.dt.float32
    Alu = mybir.AluOpType
    Act = mybir.ActivationFunctionType
    AX = mybir.AxisListType
    assert F % CW == 0
    nchunk = F // CW

    nc = bacc.Bacc(
        "TRN2",
        target_bir_lowering=False,
        debug=False,
        enable_asserts=enable_asserts,
        num_devices=N_CORES,
    )

    xs = nc.dram_tensor("xs", [n_ch, 128, F], f32, kind="ExternalInput").ap()
    w0b = nc.dram_tensor("w0b", [n_ch, 128, N_LAYERS], f32, kind="ExternalInput").ap()
    w1b = nc.dram_tensor("w1b", [n_ch, 128, N_LAYERS], f32, kind="ExternalInput").ap()
    w2b = nc.dram_tensor("w2b", [n_ch, 128, N_LAYERS], f32, kind="ExternalInput").ap()
    ys = nc.dram_tensor("ys", [n_ch, 128, F], f32, kind="ExternalOutput").ap()

    with tile.TileContext(nc) as tc:
        with (
            tc.tile_pool(name="data", bufs=2) as dpool,
            tc.tile_pool(name="scr", bufs=3) as spool,
            tc.tile_pool(name="coef", bufs=2) as cpool,
            tc.tile_pool(name="st", bufs=4) as st,
        ):

            def sbuf1(tag):
                return st.tile([128, 1], f32, tag=tag, name=tag)

            for ch in range(n_ch):
                W = dpool.tile([128, F], f32, tag="W", name="W")
                nc.sync.dma_start(out=W[:], in_=xs[ch])

                a0t = cpool.tile([128, N_LAYERS], f32, tag="a0t", name="a0t")
                a1t = cpool.tile([128, N_LAYERS], f32, tag="a1t", name="a1t")
                a2t = cpool.tile([128, N_LAYERS], f32, tag="a2t", name="a2t")
                nc.sync.dma_start(out=a0t[:], in_=w0b[ch])
                nc.sync.dma_start(out=a1t[:], in_=w1b[ch])
                nc.sync.dma_start(out=a2t[:], in_=w2b[ch])

                # sign-clamp a2: a2c = sign(a2)*max(|a2|, 1e-27), sign(0)=+1
                sgn = cpool.tile([128, N_LAYERS], f32, tag="sgn", name="sgn")
                nc.vector.tensor_scalar(sgn[:], a2t[:], 0.0, None, Alu.is_ge)
                nc.vector.tensor_scalar(sgn[:], sgn[:], 2.0, -1.0, Alu.mult, Alu.add)
                a2cl = cpool.tile([128, N_LAYERS], f32, tag="a2cl", name="a2cl")
                nc.vector.tensor_scalar(a2cl[:], a2t[:], -1.0, None, Alu.mult)
                nc.vector.tensor_tensor(a2cl[:], a2t[:], a2cl[:], Alu.max)
                nc.vector.tensor_scalar(a2cl[:], a2cl[:], 1e-27, None, Alu.max)
                nc.vector.tensor_tensor(a2cl[:], a2cl[:], sgn[:], Alu.mult)
                # delta_hat = a1/a2 for all layers at once
                dht = cpool.tile([128, N_LAYERS], f32, tag="dht", name="dht")
                nc.vector.reciprocal(dht[:], a2cl[:])
                nc.vector.tensor_tensor(dht[:], dht[:], a1t[:], Alu.mult)

                # layer-0 stats: full scans of x
                rmin = sbuf1("rmin")
                rmax = sbuf1("rmax")
                nc.vector.tensor_reduce(rmin[:], W[:], axis=AX.X, op=Alu.min)
                nc.vector.tensor_reduce(rmax[:], W[:], axis=AX.X, op=Alu.max)
                mx = sbuf1("mx")
                nc.gpsimd.partition_all_reduce(
                    mx[:], rmax[:], 128, bass_isa.ReduceOp.max
                )
                nc.vector.tensor_scalar_mul(rmin[:], rmin[:], -1.0)
                nmn = sbuf1("nmn")
                nc.gpsimd.partition_all_reduce(
                    nmn[:], rmin[:], 128, bass_isa.ReduceOp.max
                )
                mn = sbuf1("mn")
                nc.vector.tensor_scalar_mul(mn[:], nmn[:], -1.0)

                ones = sbuf1("ones")
                nc.vector.memset(ones[:], 1.0)
                rc = ones

                for l in range(N_LAYERS):
                    last = l == N_LAYERS - 1
                    a0c = a0t[:, l : l + 1]
                    a2c = a2cl[:, l : l + 1]
                    dhat = dht[:, l : l + 1]

                    # m* = g*(mn-mx) + mx, g = (rc>=0)
                    g = sbuf1("g")
                    nc.vector.tensor_scalar(g[:], rc[:], 0.0, None, Alu.is_ge)
                    dmn = sbuf1("dmn")
                    nc.vector.tensor_sub(dmn[:], mn[:], mx[:])
                    mstar = sbuf1("mstar")
                    nc.vector.tensor_mul(mstar[:], g[:], dmn[:])
                    nc.vector.tensor_add(mstar[:], mstar[:], mx[:])
                    # Delta = (mx-mn)*|rc|; s = 1/(Delta+eps); q = s*rc; A = Delta*s
                    absrc = sbuf1("absrc")
                    nc.vector.tensor_scalar_mul(absrc[:], rc[:], -1.0)
                    nc.vector.tensor_tensor(absrc[:], rc[:], absrc[:], Alu.max)
                    dW = sbuf1("dW")
                    nc.vector.tensor_sub(dW[:], mx[:], mn[:])
                    Dl = sbuf1("Dl")
                    nc.vector.tensor_mul(Dl[:], dW[:], absrc[:])
                    De = sbuf1("De")
                    nc.vector.tensor_scalar_add(De[:], Dl[:], EPS)
                    s = sbuf1("s")
                    nc.vector.reciprocal(s[:], De[:])
                    q = sbuf1("q")
                    nc.vector.tensor_mul(q[:], s[:], rc[:])
                    A = sbuf1("A")
                    nc.vector.tensor_mul(A[:], Dl[:], s[:])
                    bq = sbuf1("bq")
                    nc.vector.tensor_mul(bq[:], q[:], mstar[:])
                    nc.vector.tensor_scalar_mul(bq[:], bq[:], -1.0)

                    slots = st.tile([128, nchunk], f32, tag="slots", name="slots")
                    for k in range(nchunk):
                        Wk = W[:, k * CW : (k + 1) * CW]
                        u = spool.tile([128, CW], f32, tag="u", name="u")
                        nc.scalar.activation(
                            u[:], Wk, Act.Identity, bias=bq[:], scale=q[:]
                        )
                        s2 = spool.tile([128, CW], f32, tag="s2", name="s2")
                        nc.vector.scalar_tensor_tensor(
                            s2[:], u[:], dhat, u[:], op0=Alu.add, op1=Alu.mult
                        )
                        # copy-back with fused min-scan
                        nc.vector.tensor_scalar(
                            Wk,
                            s2[:],
                            1.0,
                            None,
                            Alu.mult,
                            Alu.min,
                            accum_out=slots[:, k : k + 1],
                        )

                    if not last:
                        # next-layer stats
                        rmn = sbuf1("rmn")
                        nc.vector.tensor_reduce(
                            rmn[:], slots[:], axis=AX.X, op=Alu.min
                        )
                        nc.vector.tensor_scalar_mul(rmn[:], rmn[:], -1.0)
                        nmn2 = sbuf1("nmn2")
                        nc.gpsimd.partition_all_reduce(
                            nmn2[:], rmn[:], 128, bass_isa.ReduceOp.max
                        )
                        mn = sbuf1("mn")
                        nc.vector.tensor_scalar_mul(mn[:], nmn2[:], -1.0)
                        # mx' = max(0, A*(A+dhat))
                        e = sbuf1("e")
                        nc.vector.tensor_scalar(e[:], A[:], dhat, None, Alu.add)
                        nc.vector.tensor_mul(e[:], e[:], A[:])
                        mx = sbuf1("mx")
                        nc.vector.tensor_scalar_max(mx[:], e[:], 0.0)
                        rc = a2c
                    else:
                        # y = a2*W' + a0
                        nc.scalar.activation(
                            W[:], W[:], Act.Identity, bias=a0c, scale=a2c
                        )

                nc.sync.dma_start(out=ys[ch], in_=W[:])

    nc.compile()
    return nc


_NC_CACHE = {}


def _get_nc():
    key = "full"
    if key not in _NC_CACHE:
        _NC_CACHE[key] = build_nc()
    return _NC_CACHE[key]


def shard_inputs(x, w0, w1, w2):
    """Full inputs -> list of per-core in_maps (channel sharding)."""
    x = np.ascontiguousarray(x, dtype=np.float32)
    in_maps = []
    for k in range(N_CORES):
        cols = slice(CH_PER_CORE * k, CH_PER_CORE * (k + 1))
        xk = np.ascontiguousarray(x[:, cols].transpose(1, 0, 2, 3)).reshape(
            CH_PER_CORE, 128, F_FULL
        )
        m = {"xs": xk}
        for nm, w in (("w0b", w0), ("w1b", w1), ("w2b", w2)):
            wc = np.asarray(w, dtype=np.float32)[:, cols]  # [20, 4]
            m[nm] = np.ascontiguousarray(
                np.broadcast_to(wc.T[:, None, :], (CH_PER_CORE, 128, N_LAYERS))
            )
        in_maps.append(m)
    return in_maps


def unshard_output(results):
    out = np.empty((B, C, H, Wd), dtype=np.float32)
    for k in range(N_CORES):
        ysk = np.asarray(results[k]["ys"], dtype=np.float32).reshape(
            CH_PER_CORE, B, H, Wd
        )
        out[:, CH_PER_CORE * k : CH_PER_CORE * (k + 1)] = ysk.transpose(1, 0, 2, 3)
    return out


def run_sharded(in_maps, trace=False, trace_kwargs=None):
    _import_concourse()
    from concourse.bass_utils import run_bass_kernel_spmd

    nc = _get_nc()
    return run_bass_kernel_spmd(
        nc,
        in_maps,
        core_ids=list(range(N_CORES)),
        trace=trace,
        **(trace_kwargs or {}),
    )


def kernel(x, w0, w1, w2):
    in_maps = shard_inputs(x, w0, w1, w2)
    res = run_sharded(in_maps)
    return unshard_output(res.results)


# revision 11
# speedup vs baseline: 1.7640x; 1.7640x over previous
"""Trainium2 Bass kernel for nn_ActSeries: 20 layers of per-channel range-norm +
quadratic polynomial, x [32,32,256,256] f32.

Strategy
--------
Shard the 32 *channels* across the 8 cores (4 channels/core). The per-layer
min/max reduction is over (B,H,W) per channel, so with channel sharding every
reduction is core-local: zero collectives. One channel is 32*256*256 floats
= 8 MB, which fits in SBUF, so each channel is loaded once, run through all
20 layers on-chip, and stored once (2 HBM passes total instead of 40+).

Math (validated vs the reference to ~1.2e-6 rel):
The stored tensor W relates to the true h by h = a2_prev * W + const (the
range-norm is invariant to this affine map, which is tracked exactly in the
[128,1] scalar chain: rc = a2_prev, rc_0 = 1). Per layer, with stats mn,mx:
  g = (rc>=0); m* = g*mn + (1-g)*mx        # stored value mapping to min h
  Delta = (mx-mn)*|rc|                     # true h range
  s = 1/(Delta+eps); q = s*rc; A = Delta*s
  u = q*W - q*m*                           # == xh, in [0,A]
  dhat = a1/a2                             # a2 sign-clamped to |a2|>=1e-27
  W' = (u + dhat)*u                        # true h' = a2*W' + a0
  stats: W' is convex in u (leading coeff 1/q^2 > 0), so
         mx' = max(0, A*(A+dhat)) from the interval endpoints (u=0 and u=A
         are attained exactly), and only the MIN needs a data scan.
Last layer: y = a2*W' + a0 in one ACT pass (scale/bias APs).

The whole per-layer data pass is ONE custom-DVE op per chunk:
  out = (Src0*C0 + C1)*(Src0*C0 + Latch(Src1));  accum_out = min(out, seed 0)
with C0=q, C1=q*(-m*)+dhat, Src1=[128,1] holding q*(-m*), i.e.
  out = (u + dhat)*u,  accum = chunk min  (seed 0 is exact: u=0 is attained)
The W buffer has F/CW + 1 chunk slots; each op reads slot k+s and writes
slot k+s-1 (mod S), so nothing is ever copied. 20 layers with S=5 returns
the data to slots 0..3.
"""

import os
import sys

import numpy as np

B, C, H, Wd = 32, 32, 256, 256
N_LAYERS = 20
EPS = 1e-5
N_CORES = 8
CH_PER_CORE = C // N_CORES  # 4
F_FULL = B * H * Wd // 128  # 16384 free-dim elements per partition


def _import_concourse():
    try:
        import concourse  # noqa: F401
    except ImportError:
        for p in ("/opt/trn_rl_repo", os.path.expanduser("~/.axon_site/_ro/trn_rl_repo")):
            if os.path.isdir(p) and p not in sys.path:
                sys.path.insert(0, p)
        import concourse  # noqa: F401


def register_fused_op():
    """Register the fused (affine)*(affine) + min-accum custom-DVE op."""
    _import_concourse()
    from concourse import dve_ops as dvo
    from concourse.dve_spec import (
        C0,
        C1,
        C3,
        AluOp,
        Spec,
        Src0,
        Zero,
        _has_src1,
        _spill_c3_to_src1,
        lower,
    )
    from concourse.dve_uop import DveOpSpec

    name = "RANGE_POLY_MIN_ANT"
    for op in dvo.OPS:
        if op.name == name:
            return op

    def _ref(in0, in1, s0, s1, imm2):
        x = in0.astype(np.float32)
        b2 = np.asarray(in1, dtype=np.float32).reshape(x.shape[0], -1)[:, :1]
        t = (x * s0).astype(np.float32)
        o = ((t + s1) * (t + b2)).astype(np.float32)
        acc = np.minimum(
            o.reshape(o.shape[0], -1).min(axis=-1, keepdims=True), np.float32(0.0)
        ).astype(np.float32)
        return o, acc

    t = Src0 * C0
    body = _spill_c3_to_src1((t + C1) * (t + C3))
    spec = Spec(body=body, accum=AluOp.MIN, accum_init=Zero, reference=_ref)
    row = max(dvo._SUB_OPCODE_FOR_NAME.values()) + 1
    uops = lower(spec, ver="v3")
    sha = DveOpSpec(name=name, opcode=row, uops=uops, rd1_en=_has_src1(spec)).sha("v3")
    op = dvo.DveOp(name=name, spec=spec, subdim=False, uops_sha={"v3": sha})
    dvo.OPS.append(op)
    dvo._SUB_OPCODE_FOR_NAME[name] = row
    dvo.CUSTOM_DVE_SPECS[name] = spec
    return op


def build_nc(F=F_FULL, CW=4096, n_ch=CH_PER_CORE, enable_asserts=False):
    """Build the (single, SPMD) Bass program. Returns the compiled nc."""
    _import_concourse()
    import concourse.bacc as bacc
    import concourse.tile as tile
    from concourse import bass_isa, mybir

    fused = register_fused_op()

    f32 = mybir.dt.float32
    Alu = mybir.AluOpType
    Act = mybir.ActivationFunctionType
    AX = mybir.AxisListType
    assert F % CW == 0
    nchunk = F // CW
    S = nchunk + 1  # rotation slots
    assert (N_LAYERS % S) == 0, "layer count must return data to slot 0"

    nc = bacc.Bacc(
        "TRN2",
        target_bir_lowering=False,
        debug=False,
        enable_asserts=enable_asserts,
        num_devices=N_CORES,
    )

    xs = nc.dram_tensor("xs", [n_ch, 128, F], f32, kind="ExternalInput").ap()
    w0b = nc.dram_tensor("w0b", [n_ch, 128, N_LAYERS], f32, kind="ExternalInput").ap()
    w1b = nc.dram_tensor("w1b", [n_ch, 128, N_LAYERS], f32, kind="ExternalInput").ap()
    w2b = nc.dram_tensor("w2b", [n_ch, 128, N_LAYERS], f32, kind="ExternalInput").ap()
    ys = nc.dram_tensor("ys", [n_ch, 128, F], f32, kind="ExternalOutput").ap()

    with tile.TileContext(nc) as tc:
        with (
            tc.tile_pool(name="data", bufs=2) as dpool,
            tc.tile_pool(name="coef", bufs=2) as cpool,
            tc.tile_pool(name="st", bufs=4) as st,
        ):

            def sbuf1(tag):
                return st.tile([128, 1], f32, tag=tag, name=tag)

            for ch in range(n_ch):
                W = dpool.tile([128, S * CW], f32, tag="W", name="W")
                nc.sync.dma_start(out=W[:, 0:F], in_=xs[ch])

                a0t = cpool.tile([128, N_LAYERS], f32, tag="a0t", name="a0t")
                a1t = cpool.tile([128, N_LAYERS], f32, tag="a1t", name="a1t")
                a2t = cpool.tile([128, N_LAYERS], f32, tag="a2t", name="a2t")
                nc.sync.dma_start(out=a0t[:], in_=w0b[ch])
                nc.sync.dma_start(out=a1t[:], in_=w1b[ch])
                nc.sync.dma_start(out=a2t[:], in_=w2b[ch])

                # sign-clamp a2: a2cl = sign(a2)*max(|a2|, 1e-27), sign(0)=+1
                sgn = cpool.tile([128, N_LAYERS], f32, tag="sgn", name="sgn")
                nc.vector.tensor_scalar(sgn[:], a2t[:], 0.0, None, Alu.is_ge)
                nc.vector.tensor_scalar(sgn[:], sgn[:], 2.0, -1.0, Alu.mult, Alu.add)
                a2cl = cpool.tile([128, N_LAYERS], f32, tag="a2cl", name="a2cl")
                nc.vector.tensor_scalar(a2cl[:], a2t[:], -1.0, None, Alu.mult)
                nc.vector.tensor_tensor(a2cl[:], a2t[:], a2cl[:], Alu.max)
                nc.vector.tensor_scalar(a2cl[:], a2cl[:], 1e-27, None, Alu.max)
                nc.vector.tensor_tensor(a2cl[:], a2cl[:], sgn[:], Alu.mult)
                # dhat = a1/a2 for all layers at once
                dht = cpool.tile([128, N_LAYERS], f32, tag="dht", name="dht")
                nc.vector.reciprocal(dht[:], a2cl[:])
                nc.vector.tensor_tensor(dht[:], dht[:], a1t[:], Alu.mult)

                # layer-0 stats: full scans of x
                rmin = sbuf1("rmin")
                rmax = sbuf1("rmax")
                nc.vector.tensor_reduce(rmin[:], W[:, 0:F], axis=AX.X, op=Alu.min)
                nc.vector.tensor_reduce(rmax[:], W[:, 0:F], axis=AX.X, op=Alu.max)
                mx = sbuf1("mx")
                nc.gpsimd.partition_all_reduce(
                    mx[:], rmax[:], 128, bass_isa.ReduceOp.max
                )
                nc.scalar.mul(rmin[:], rmin[:], -1.0)
                nmn = sbuf1("nmn")
                nc.gpsimd.partition_all_reduce(
                    nmn[:], rmin[:], 128, bass_isa.ReduceOp.max
                )
                mn = sbuf1("mn")
                nc.scalar.mul(mn[:], nmn[:], -1.0)

                ones = sbuf1("ones")
                nc.vector.memset(ones[:], 1.0)
                rc = ones

                for l in range(N_LAYERS):
                    last = l == N_LAYERS - 1
                    a0c = a0t[:, l : l + 1]
                    a2c = a2cl[:, l : l + 1]
                    dhat = dht[:, l : l + 1]
                    src = (-l) % S  # slot of chunk 0 this layer

                    # m* = g*(mn-mx) + mx, g = (rc>=0)
                    g = sbuf1("g")
                    nc.vector.tensor_scalar(g[:], rc[:], 0.0, None, Alu.is_ge)
                    dmn = sbuf1("dmn")
                    nc.vector.tensor_sub(dmn[:], mn[:], mx[:])
                    mstar = sbuf1("mstar")
                    nc.vector.tensor_mul(mstar[:], g[:], dmn[:])
                    nc.vector.tensor_add(mstar[:], mstar[:], mx[:])
                    # Delta = (mx-mn)*|rc|; s = 1/(Delta+eps); q = s*rc; A = Delta*s
                    absrc = sbuf1("absrc")
                    nc.scalar.mul(absrc[:], rc[:], -1.0)
                    nc.vector.tensor_tensor(absrc[:], rc[:], absrc[:], Alu.max)
                    dW = sbuf1("dW")
                    nc.vector.tensor_sub(dW[:], mx[:], mn[:])
                    Dl = sbuf1("Dl")
                    nc.vector.tensor_mul(Dl[:], dW[:], absrc[:])
                    De = sbuf1("De")
                    nc.vector.tensor_scalar_add(De[:], Dl[:], EPS)
                    s = sbuf1("s")
                    nc.vector.reciprocal(s[:], De[:])
                    q = sbuf1("q")
                    nc.vector.tensor_mul(q[:], s[:], rc[:])
                    A = sbuf1("A")
                    nc.vector.tensor_mul(A[:], Dl[:], s[:])
                    # b = -(q*m*); btl = b + dhat
                    bq = sbuf1("bq")
                    nc.vector.tensor_mul(bq[:], q[:], mstar[:])
                    nc.scalar.mul(bq[:], bq[:], -1.0)
                    btl = sbuf1("btl")
                    nc.vector.tensor_add(btl[:], bq[:], dhat)

                    slots = st.tile([128, nchunk], f32, tag="slots", name="slots")
                    for k in range(nchunk):
                        rd = W[:, ((k + src) % S) * CW :][:, :CW]
                        wr = W[:, ((k + src - 1) % S) * CW :][:, :CW]
                        nc.vector._custom_dve(
                            fused,
                            out=wr,
                            in0=rd,
                            in1=bq[:],
                            s0=q[:],
                            s1=btl[:],
                            accum_out=slots[:, k : k + 1],
                        )

                    if not last:
                        # next-layer stats
                        rmn = sbuf1("rmn")
                        nc.vector.tensor_reduce(
                            rmn[:], slots[:], axis=AX.X, op=Alu.min
                        )
                        nc.scalar.mul(rmn[:], rmn[:], -1.0)
                        nmn2 = sbuf1("nmn2")
                        nc.gpsimd.partition_all_reduce(
                            nmn2[:], rmn[:], 128, bass_isa.ReduceOp.max
                        )
                        mn = sbuf1("mn")
                        nc.scalar.mul(mn[:], nmn2[:], -1.0)
                        # mx' = max(0, A*(A+dhat))
                        e = sbuf1("e")
                        nc.vector.tensor_scalar(e[:], A[:], dhat, None, Alu.add)
                        nc.vector.tensor_mul(e[:], e[:], A[:])
                        mx = sbuf1("mx")
                        nc.vector.tensor_scalar_max(mx[:], e[:], 0.0)
                        rc = a2c
                    else:
                        # y = a2*W' + a0  (data is back at slots 0..nchunk-1)
                        nc.scalar.activation(
                            W[:, 0:F], W[:, 0:F], Act.Identity, bias=a0c, scale=a2c
                        )

                nc.sync.dma_start(out=ys[ch], in_=W[:, 0:F])

    nc.compile()
    return nc


_NC_CACHE = {}


def _get_nc():
    key = "full"
    if key not in _NC_CACHE:
        _NC_CACHE[key] = build_nc()
    return _NC_CACHE[key]


def shard_inputs(x, w0, w1, w2):
    """Full inputs -> list of per-core in_maps (channel sharding)."""
    x = np.ascontiguousarray(x, dtype=np.float32)
    in_maps = []
    for k in range(N_CORES):
        cols = slice(CH_PER_CORE * k, CH_PER_CORE * (k + 1))
        xk = np.ascontiguousarray(x[:, cols].transpose(1, 0, 2, 3)).reshape(
            CH_PER_CORE, 128, F_FULL
        )
        m = {"xs": xk}
        for nm, w in (("w0b", w0), ("w1b", w1), ("w2b", w2)):
            wc = np.asarray(w, dtype=np.float32)[:, cols]  # [20, 4]
            m[nm] = np.ascontiguousarray(
                np.broadcast_to(wc.T[:, None, :], (CH_PER_CORE, 128, N_LAYERS))
            )
        in_maps.append(m)
    return in_maps


def unshard_output(results):
    out = np.empty((B, C, H, Wd), dtype=np.float32)
    for k in range(N_CORES):
        ysk = np.asarray(results[k]["ys"], dtype=np.float32).reshape(
            CH_PER_CORE, B, H, Wd
        )
        out[:, CH_PER_CORE * k : CH_PER_CORE * (k + 1)] = ysk.transpose(1, 0, 2, 3)
    return out


def run_sharded(in_maps, trace=False, trace_kwargs=None):
    _import_concourse()
    from concourse.bass_utils import run_bass_kernel_spmd

    nc = _get_nc()
    return run_bass_kernel_spmd(
        nc,
        in_maps,
        core_ids=list(range(N_CORES)),
        trace=trace,
        **(trace_kwargs or {}),
    )


def kernel(x, w0, w1, w2):
    in_maps = shard_inputs(x, w0, w1, w2)
    res = run_sharded(in_maps)
    return unshard_output(res.results)


# revision 14
# speedup vs baseline: 1.8453x; 1.0461x over previous
"""Trainium2 Bass kernel for nn_ActSeries: 20 layers of per-channel range-norm +
quadratic polynomial, x [32,32,256,256] f32.

Strategy
--------
Shard the 32 *channels* across the 8 cores (4 channels/core). The per-layer
min/max reduction is over (B,H,W) per channel, so with channel sharding every
reduction is core-local: zero collectives. One channel is 32*256*256 floats
= 8 MB, which fits in SBUF, so each channel is loaded once, run through all
20 layers on-chip, and stored once (2 HBM passes total instead of 40+).

Math (validated vs the reference to ~1.2e-6 rel):
The stored tensor W relates to the true h by h = a2_prev * W + const (the
range-norm is invariant to this affine map, which is tracked exactly in the
[128,1] scalar chain: rc = a2_prev, rc_0 = 1). Per layer, with stats mn,mx:
  g = (rc>=0); m* = g*mn + (1-g)*mx        # stored value mapping to min h
  Delta = (mx-mn)*|rc|                     # true h range
  s = 1/(Delta+eps); q = s*rc; A = Delta*s
  u = q*W - q*m*                           # == xh, in [0,A]
  dhat = a1/a2                             # a2 sign-clamped to |a2|>=1e-27
  W' = (u + dhat)*u                        # true h' = a2*W' + a0
  stats: W' is convex in u (leading coeff 1/q^2 > 0), so
         mx' = max(0, A*(A+dhat)) from the interval endpoints (u=0 and u=A
         are attained exactly), and only the MIN needs a data scan.
Last layer: y = a2*W' + a0 in one ACT pass (scale/bias APs).

The whole per-layer data pass is ONE custom-DVE op per chunk:
  out = (Src0*C0 + C1)*(Src0*C0 + Latch(Src1));  accum_out = min(out, seed 0)
with C0=q, C1=q*(-m*)+dhat, Src1=[128,1] holding q*(-m*), i.e.
  out = (u + dhat)*u,  accum = chunk min  (seed 0 is exact: u=0 is attained)
The W buffer has F/CW + 1 chunk slots; each op reads slot k+s and writes
slot k+s-1 (mod S), so nothing is ever copied. 20 layers with S=5 returns
the data to slots 0..3.
"""

import os
import sys

import numpy as np

B, C, H, Wd = 32, 32, 256, 256
N_LAYERS = 20
EPS = 1e-5
N_CORES = 8
CH_PER_CORE = C // N_CORES  # 4
F_FULL = B * H * Wd // 128  # 16384 free-dim elements per partition


def _import_concourse():
    try:
        import concourse  # noqa: F401
    except ImportError:
        for p in ("/opt/trn_rl_repo", os.path.expanduser("~/.axon_site/_ro/trn_rl_repo")):
            if os.path.isdir(p) and p not in sys.path:
                sys.path.insert(0, p)
        import concourse  # noqa: F401


def register_fused_op():
    """Register the fused (affine)*(affine) + min-accum custom-DVE op."""
    _import_concourse()
    from concourse import dve_ops as dvo
    from concourse.dve_spec import (
        C0,
        C1,
        C3,
        AluOp,
        Spec,
        Src0,
        Zero,
        _has_src1,
        _spill_c3_to_src1,
        lower,
    )
    from concourse.dve_uop import DveOpSpec

    name = "RANGE_POLY_MIN_ANT"
    for op in dvo.OPS:
        if op.name == name:
            return op

    def _ref(in0, in1, s0, s1, imm2):
        x = in0.astype(np.float32)
        b2 = np.asarray(in1, dtype=np.float32).reshape(x.shape[0], -1)[:, :1]
        t = (x * s0).astype(np.float32)
        o = ((t + s1) * (t + b2)).astype(np.float32)
        acc = np.minimum(
            o.reshape(o.shape[0], -1).min(axis=-1, keepdims=True), np.float32(0.0)
        ).astype(np.float32)
        return o, acc

    t = Src0 * C0
    body = _spill_c3_to_src1((t + C1) * (t + C3))
    spec = Spec(body=body, accum=AluOp.MIN, accum_init=Zero, reference=_ref)
    row = max(dvo._SUB_OPCODE_FOR_NAME.values()) + 1
    uops = lower(spec, ver="v3")
    sha = DveOpSpec(name=name, opcode=row, uops=uops, rd1_en=_has_src1(spec)).sha("v3")
    op = dvo.DveOp(name=name, spec=spec, subdim=False, uops_sha={"v3": sha})
    dvo.OPS.append(op)
    dvo._SUB_OPCODE_FOR_NAME[name] = row
    dvo.CUSTOM_DVE_SPECS[name] = spec
    return op


def build_nc(F=F_FULL, CW=4096, n_ch=CH_PER_CORE, enable_asserts=False):
    """Build the (single, SPMD) Bass program. Returns the compiled nc."""
    _import_concourse()
    import concourse.bacc as bacc
    import concourse.tile as tile
    from concourse import bass_isa, mybir

    fused = register_fused_op()

    f32 = mybir.dt.float32
    Alu = mybir.AluOpType
    Act = mybir.ActivationFunctionType
    AX = mybir.AxisListType
    assert F % CW == 0
    nchunk = F // CW
    S = nchunk + 1  # rotation slots
    assert (N_LAYERS % S) == 0, "layer count must return data to slot 0"

    nc = bacc.Bacc(
        "TRN2",
        target_bir_lowering=False,
        debug=False,
        enable_asserts=enable_asserts,
        num_devices=N_CORES,
    )

    xs = nc.dram_tensor("xs", [n_ch, 128, F], f32, kind="ExternalInput").ap()
    w0b = nc.dram_tensor("w0b", [n_ch, 128, N_LAYERS], f32, kind="ExternalInput").ap()
    w1b = nc.dram_tensor("w1b", [n_ch, 128, N_LAYERS], f32, kind="ExternalInput").ap()
    w2b = nc.dram_tensor("w2b", [n_ch, 128, N_LAYERS], f32, kind="ExternalInput").ap()
    ys = nc.dram_tensor("ys", [n_ch, 128, F], f32, kind="ExternalOutput").ap()

    with tile.TileContext(nc) as tc:
        with (
            tc.tile_pool(name="data", bufs=2) as dpool,
            tc.tile_pool(name="coef", bufs=2) as cpool,
            tc.tile_pool(name="st", bufs=4) as st,
        ):

            def sbuf1(tag):
                return st.tile([128, 1], f32, tag=tag, name=tag)

            for ch in range(n_ch):
                W = dpool.tile([128, S * CW], f32, tag="W", name="W")
                nc.sync.dma_start(out=W[:, 0:F], in_=xs[ch])

                a0t = cpool.tile([128, N_LAYERS], f32, tag="a0t", name="a0t")
                a1t = cpool.tile([128, N_LAYERS], f32, tag="a1t", name="a1t")
                a2t = cpool.tile([128, N_LAYERS], f32, tag="a2t", name="a2t")
                nc.sync.dma_start(out=a0t[:], in_=w0b[ch])
                nc.sync.dma_start(out=a1t[:], in_=w1b[ch])
                nc.sync.dma_start(out=a2t[:], in_=w2b[ch])

                # sign-clamp a2: a2cl = sign(a2)*max(|a2|, 1e-27), sign(0)=+1
                sgn = cpool.tile([128, N_LAYERS], f32, tag="sgn", name="sgn")
                nc.vector.tensor_scalar(sgn[:], a2t[:], 0.0, None, Alu.is_ge)
                nc.vector.tensor_scalar(sgn[:], sgn[:], 2.0, -1.0, Alu.mult, Alu.add)
                a2cl = cpool.tile([128, N_LAYERS], f32, tag="a2cl", name="a2cl")
                nc.vector.tensor_scalar(a2cl[:], a2t[:], -1.0, None, Alu.mult)
                nc.vector.tensor_tensor(a2cl[:], a2t[:], a2cl[:], Alu.max)
                nc.vector.tensor_scalar(a2cl[:], a2cl[:], 1e-27, None, Alu.max)
                nc.vector.tensor_tensor(a2cl[:], a2cl[:], sgn[:], Alu.mult)
                # dhat = a1/a2 for all layers at once
                dht = cpool.tile([128, N_LAYERS], f32, tag="dht", name="dht")
                nc.vector.reciprocal(dht[:], a2cl[:])
                nc.vector.tensor_tensor(dht[:], dht[:], a1t[:], Alu.mult)

                # layer-0 stats: full scans of x
                rmin = sbuf1("rmin")
                rmax = sbuf1("rmax")
                nc.vector.tensor_reduce(rmin[:], W[:, 0:F], axis=AX.X, op=Alu.min)
                nc.vector.tensor_reduce(rmax[:], W[:, 0:F], axis=AX.X, op=Alu.max)
                mx = sbuf1("mx")
                nc.gpsimd.partition_all_reduce(
                    mx[:], rmax[:], 128, bass_isa.ReduceOp.max
                )
                nc.vector.tensor_scalar_mul(rmin[:], rmin[:], -1.0)
                nmn = sbuf1("nmn")
                nc.gpsimd.partition_all_reduce(
                    nmn[:], rmin[:], 128, bass_isa.ReduceOp.max
                )
                mn = sbuf1("mn")
                nc.vector.tensor_scalar_mul(mn[:], nmn[:], -1.0)

                ones = sbuf1("ones")
                nc.vector.memset(ones[:], 1.0)
                rc = ones

                for l in range(N_LAYERS):
                    last = l == N_LAYERS - 1
                    a0c = a0t[:, l : l + 1]
                    a2c = a2cl[:, l : l + 1]
                    dhat = dht[:, l : l + 1]
                    src = (-l) % S  # slot of chunk 0 this layer

                    # m* = g*(mn-mx) + mx, g = (rc>=0)
                    g = sbuf1("g")
                    nc.vector.tensor_scalar(g[:], rc[:], 0.0, None, Alu.is_ge)
                    dmn = sbuf1("dmn")
                    nc.vector.tensor_sub(dmn[:], mn[:], mx[:])
                    mstar = sbuf1("mstar")
                    nc.vector.tensor_mul(mstar[:], g[:], dmn[:])
                    nc.vector.tensor_add(mstar[:], mstar[:], mx[:])
                    # Delta = (mx-mn)*|rc|; s = 1/(Delta+eps); q = s*rc; A = Delta*s
                    absrc = sbuf1("absrc")
                    nc.vector.tensor_scalar_mul(absrc[:], rc[:], -1.0)
                    nc.vector.tensor_tensor(absrc[:], rc[:], absrc[:], Alu.max)
                    dW = sbuf1("dW")
                    nc.vector.tensor_sub(dW[:], mx[:], mn[:])
                    Dl = sbuf1("Dl")
                    nc.vector.tensor_mul(Dl[:], dW[:], absrc[:])
                    De = sbuf1("De")
                    nc.vector.tensor_scalar_add(De[:], Dl[:], EPS)
                    s = sbuf1("s")
                    nc.vector.reciprocal(s[:], De[:])
                    q = sbuf1("q")
                    nc.vector.tensor_mul(q[:], s[:], rc[:])
                    A = sbuf1("A")
                    nc.vector.tensor_mul(A[:], Dl[:], s[:])
                    # b = -(q*m*); btl = b + dhat
                    bq = sbuf1("bq")
                    nc.vector.tensor_mul(bq[:], q[:], mstar[:])
                    nc.vector.tensor_scalar_mul(bq[:], bq[:], -1.0)
                    btl = sbuf1("btl")
                    nc.vector.tensor_add(btl[:], bq[:], dhat)
                    if not last:
                        # endpoint max for next layer: mx' = max(0, A*(A+dhat))
                        # (independent of the scan - compute it up front)
                        e = sbuf1("e")
                        nc.vector.tensor_scalar(e[:], A[:], dhat, None, Alu.add)
                        nc.vector.tensor_mul(e[:], e[:], A[:])
                        mx = sbuf1("mx")
                        nc.vector.tensor_scalar_max(mx[:], e[:], 0.0)

                    slots = st.tile([128, nchunk], f32, tag="slots", name="slots")
                    for k in range(nchunk):
                        rd = W[:, ((k + src) % S) * CW :][:, :CW]
                        wr = W[:, ((k + src - 1) % S) * CW :][:, :CW]
                        nc.vector._custom_dve(
                            fused,
                            out=wr,
                            in0=rd,
                            in1=bq[:],
                            s0=q[:],
                            s1=btl[:],
                            accum_out=slots[:, k : k + 1],
                        )

                    if not last:
                        # next-layer stats
                        rmn = sbuf1("rmn")
                        nc.vector.tensor_reduce(
                            rmn[:], slots[:], axis=AX.X, op=Alu.min
                        )
                        nc.vector.tensor_scalar_mul(rmn[:], rmn[:], -1.0)
                        nmn2 = sbuf1("nmn2")
                        nc.gpsimd.partition_all_reduce(
                            nmn2[:], rmn[:], 128, bass_isa.ReduceOp.max
                        )
                        mn = sbuf1("mn")
                        nc.vector.tensor_scalar_mul(mn[:], nmn2[:], -1.0)
                        rc = a2c
                    else:
                        # y = a2*W' + a0, chunked so the store overlaps
                        # (data is back at slots 0..nchunk-1)
                        for k in range(nchunk):
                            seg = W[:, k * CW : (k + 1) * CW]
                            nc.scalar.activation(
                                seg, seg, Act.Identity, bias=a0c, scale=a2c
                            )
                            nc.sync.dma_start(
                                out=ys[ch][:, k * CW : (k + 1) * CW], in_=seg
                            )

    nc.compile()
    return nc


_NC_CACHE = {}


def _get_nc():
    key = "full"
    if key not in _NC_CACHE:
        _NC_CACHE[key] = build_nc()
    return _NC_CACHE[key]


def shard_inputs(x, w0, w1, w2):
    """Full inputs -> list of per-core in_maps (channel sharding)."""
    x = np.ascontiguousarray(x, dtype=np.float32)
    in_maps = []
    for k in range(N_CORES):
        cols = slice(CH_PER_CORE * k, CH_PER_CORE * (k + 1))
        xk = np.ascontiguousarray(x[:, cols].transpose(1, 0, 2, 3)).reshape(
            CH_PER_CORE, 128, F_FULL
        )
        m = {"xs": xk}
        for nm, w in (("w0b", w0), ("w1b", w1), ("w2b", w2)):
            wc = np.asarray(w, dtype=np.float32)[:, cols]  # [20, 4]
            m[nm] = np.ascontiguousarray(
                np.broadcast_to(wc.T[:, None, :], (CH_PER_CORE, 128, N_LAYERS))
            )
        in_maps.append(m)
    return in_maps


def unshard_output(results):
    out = np.empty((B, C, H, Wd), dtype=np.float32)
    for k in range(N_CORES):
        ysk = np.asarray(results[k]["ys"], dtype=np.float32).reshape(
            CH_PER_CORE, B, H, Wd
        )
        out[:, CH_PER_CORE * k : CH_PER_CORE * (k + 1)] = ysk.transpose(1, 0, 2, 3)
    return out


def run_sharded(in_maps, trace=False, trace_kwargs=None):
    _import_concourse()
    from concourse.bass_utils import run_bass_kernel_spmd

    nc = _get_nc()
    return run_bass_kernel_spmd(
        nc,
        in_maps,
        core_ids=list(range(N_CORES)),
        trace=trace,
        **(trace_kwargs or {}),
    )


def kernel(x, w0, w1, w2):
    in_maps = shard_inputs(x, w0, w1, w2)
    res = run_sharded(in_maps)
    return unshard_output(res.results)


# revision 16
# speedup vs baseline: 1.8464x; 1.0006x over previous
"""Trainium2 Bass kernel for nn_ActSeries: 20 layers of per-channel range-norm +
quadratic polynomial, x [32,32,256,256] f32.

Strategy
--------
Shard the 32 *channels* across the 8 cores (4 channels/core). The per-layer
min/max reduction is over (B,H,W) per channel, so with channel sharding every
reduction is core-local: zero collectives. One channel is 32*256*256 floats
= 8 MB, which fits in SBUF, so each channel is loaded once, run through all
20 layers on-chip, and stored once (2 HBM passes total instead of 40+).

Math (validated vs the reference to ~1.2e-6 rel):
The stored tensor W relates to the true h by h = a2_prev * W + const (the
range-norm is invariant to this affine map, which is tracked exactly in the
[128,1] scalar chain: rc = a2_prev, rc_0 = 1). Per layer, with stats mn,mx:
  g = (rc>=0); m* = g*mn + (1-g)*mx        # stored value mapping to min h
  Delta = (mx-mn)*|rc|                     # true h range
  s = 1/(Delta+eps); q = s*rc; A = Delta*s
  u = q*W - q*m*                           # == xh, in [0,A]
  dhat = a1/a2                             # a2 sign-clamped to |a2|>=1e-27
  W' = (u + dhat)*u                        # true h' = a2*W' + a0
  stats: W' is convex in u (leading coeff 1/q^2 > 0), so
         mx' = max(0, A*(A+dhat)) from the interval endpoints (u=0 and u=A
         are attained exactly), and only the MIN needs a data scan.
Last layer: y = a2*W' + a0 in one ACT pass (scale/bias APs).

The whole per-layer data pass is ONE custom-DVE op per chunk:
  out = (Src0*C0 + C1)*(Src0*C0 + Latch(Src1));  accum_out = min(out, seed 0)
with C0=q, C1=q*(-m*)+dhat, Src1=[128,1] holding q*(-m*), i.e.
  out = (u + dhat)*u,  accum = chunk min  (seed 0 is exact: u=0 is attained)
The W buffer has F/CW + 1 chunk slots; each op reads slot k+s and writes
slot k+s-1 (mod S), so nothing is ever copied. 20 layers with S=5 returns
the data to slots 0..3.
"""

import os
import sys

import numpy as np

B, C, H, Wd = 32, 32, 256, 256
N_LAYERS = 20
EPS = 1e-5
N_CORES = 8
CH_PER_CORE = C // N_CORES  # 4
F_FULL = B * H * Wd // 128  # 16384 free-dim elements per partition


def _import_concourse():
    try:
        import concourse  # noqa: F401
    except ImportError:
        for p in ("/opt/trn_rl_repo", os.path.expanduser("~/.axon_site/_ro/trn_rl_repo")):
            if os.path.isdir(p) and p not in sys.path:
                sys.path.insert(0, p)
        import concourse  # noqa: F401


def register_fused_op():
    """Register the fused (affine)*(affine) + min-accum custom-DVE op."""
    _import_concourse()
    from concourse import dve_ops as dvo
    from concourse.dve_spec import (
        C0,
        C1,
        C3,
        AluOp,
        Spec,
        Src0,
        Zero,
        _has_src1,
        _spill_c3_to_src1,
        lower,
    )
    from concourse.dve_uop import DveOpSpec

    name = "RANGE_POLY_MIN_ANT"
    for op in dvo.OPS:
        if op.name == name:
            return op

    def _ref(in0, in1, s0, s1, imm2):
        x = in0.astype(np.float32)
        b2 = np.asarray(in1, dtype=np.float32).reshape(x.shape[0], -1)[:, :1]
        t = (x * s0).astype(np.float32)
        o = ((t + s1) * (t + b2)).astype(np.float32)
        acc = np.minimum(
            o.reshape(o.shape[0], -1).min(axis=-1, keepdims=True), np.float32(0.0)
        ).astype(np.float32)
        return o, acc

    t = Src0 * C0
    body = _spill_c3_to_src1((t + C1) * (t + C3))
    spec = Spec(body=body, accum=AluOp.MIN, accum_init=Zero, reference=_ref)
    row = max(dvo._SUB_OPCODE_FOR_NAME.values()) + 1
    uops = lower(spec, ver="v3")
    sha = DveOpSpec(name=name, opcode=row, uops=uops, rd1_en=_has_src1(spec)).sha("v3")
    op = dvo.DveOp(name=name, spec=spec, subdim=False, uops_sha={"v3": sha})
    dvo.OPS.append(op)
    dvo._SUB_OPCODE_FOR_NAME[name] = row
    dvo.CUSTOM_DVE_SPECS[name] = spec
    return op


def build_nc(F=F_FULL, CW=4096, n_ch=CH_PER_CORE, enable_asserts=False):
    """Build the (single, SPMD) Bass program. Returns the compiled nc."""
    _import_concourse()
    import concourse.bacc as bacc
    import concourse.tile as tile
    from concourse import bass_isa, mybir

    fused = register_fused_op()

    f32 = mybir.dt.float32
    Alu = mybir.AluOpType
    Act = mybir.ActivationFunctionType
    AX = mybir.AxisListType
    assert F % CW == 0
    nchunk = F // CW
    S = nchunk + 1  # rotation slots
    assert (N_LAYERS % S) == 0, "layer count must return data to slot 0"

    nc = bacc.Bacc(
        "TRN2",
        target_bir_lowering=False,
        debug=False,
        enable_asserts=enable_asserts,
        num_devices=N_CORES,
    )

    xs = nc.dram_tensor("xs", [n_ch, 128, F], f32, kind="ExternalInput").ap()
    w0b = nc.dram_tensor("w0b", [n_ch, 128, N_LAYERS], f32, kind="ExternalInput").ap()
    w1b = nc.dram_tensor("w1b", [n_ch, 128, N_LAYERS], f32, kind="ExternalInput").ap()
    w2b = nc.dram_tensor("w2b", [n_ch, 128, N_LAYERS], f32, kind="ExternalInput").ap()
    ys = nc.dram_tensor("ys", [n_ch, 128, F], f32, kind="ExternalOutput").ap()

    with tile.TileContext(nc) as tc:
        with (
            tc.tile_pool(name="data", bufs=2) as dpool,
            tc.tile_pool(name="coef", bufs=2) as cpool,
            tc.tile_pool(name="st", bufs=4) as st,
        ):

            def sbuf1(tag):
                return st.tile([128, 1], f32, tag=tag, name=tag)

            for ch in range(n_ch):
                W = dpool.tile([128, S * CW], f32, tag="W", name="W")
                nc.sync.dma_start(out=W[:, 0:F], in_=xs[ch])

                a0t = cpool.tile([128, N_LAYERS], f32, tag="a0t", name="a0t")
                a1t = cpool.tile([128, N_LAYERS], f32, tag="a1t", name="a1t")
                a2t = cpool.tile([128, N_LAYERS], f32, tag="a2t", name="a2t")
                nc.sync.dma_start(out=a0t[:], in_=w0b[ch])
                nc.sync.dma_start(out=a1t[:], in_=w1b[ch])
                nc.sync.dma_start(out=a2t[:], in_=w2b[ch])

                # sign-clamp a2: a2cl = sign(a2)*max(|a2|, 1e-27), sign(0)=+1
                sgn = cpool.tile([128, N_LAYERS], f32, tag="sgn", name="sgn")
                nc.vector.tensor_scalar(sgn[:], a2t[:], 0.0, None, Alu.is_ge)
                nc.vector.tensor_scalar(sgn[:], sgn[:], 2.0, -1.0, Alu.mult, Alu.add)
                a2cl = cpool.tile([128, N_LAYERS], f32, tag="a2cl", name="a2cl")
                nc.vector.tensor_scalar(a2cl[:], a2t[:], -1.0, None, Alu.mult)
                nc.vector.tensor_tensor(a2cl[:], a2t[:], a2cl[:], Alu.max)
                nc.vector.tensor_scalar(a2cl[:], a2cl[:], 1e-27, None, Alu.max)
                nc.vector.tensor_tensor(a2cl[:], a2cl[:], sgn[:], Alu.mult)
                # dhat = a1/a2 for all layers at once
                dht = cpool.tile([128, N_LAYERS], f32, tag="dht", name="dht")
                nc.vector.reciprocal(dht[:], a2cl[:])
                nc.vector.tensor_tensor(dht[:], dht[:], a1t[:], Alu.mult)
                # per-layer rc = a2cl[l-1] (rc_0 = 1): batch-precompute
                # g_all = (rc>=0) and absrc_all = |rc| for every layer
                g_all = cpool.tile([128, N_LAYERS], f32, tag="g_all", name="g_all")
                nc.vector.memset(g_all[:, 0:1], 1.0)
                nc.vector.tensor_scalar(
                    g_all[:, 1:], a2cl[:, : N_LAYERS - 1], 0.0, None, Alu.is_ge
                )
                absrc_all = cpool.tile(
                    [128, N_LAYERS], f32, tag="absrc_all", name="absrc_all"
                )
                nc.vector.memset(absrc_all[:, 0:1], 1.0)
                nc.vector.tensor_scalar(
                    absrc_all[:, 1:], a2cl[:, : N_LAYERS - 1], -1.0, None, Alu.mult
                )
                nc.vector.tensor_tensor(
                    absrc_all[:, 1:], a2cl[:, : N_LAYERS - 1], absrc_all[:, 1:], Alu.max
                )

                # layer-0 stats: full scans of x
                rmin = sbuf1("rmin")
                rmax = sbuf1("rmax")
                nc.vector.tensor_reduce(rmin[:], W[:, 0:F], axis=AX.X, op=Alu.min)
                nc.vector.tensor_reduce(rmax[:], W[:, 0:F], axis=AX.X, op=Alu.max)
                mx = sbuf1("mx")
                nc.gpsimd.partition_all_reduce(
                    mx[:], rmax[:], 128, bass_isa.ReduceOp.max
                )
                nc.vector.tensor_scalar_mul(rmin[:], rmin[:], -1.0)
                nmn = sbuf1("nmn")
                nc.gpsimd.partition_all_reduce(
                    nmn[:], rmin[:], 128, bass_isa.ReduceOp.max
                )
                mn = sbuf1("mn")
                nc.vector.tensor_scalar_mul(mn[:], nmn[:], -1.0)

                ones = sbuf1("ones")
                nc.vector.memset(ones[:], 1.0)
                rc = ones

                for l in range(N_LAYERS):
                    last = l == N_LAYERS - 1
                    a0c = a0t[:, l : l + 1]
                    a2c = a2cl[:, l : l + 1]
                    dhat = dht[:, l : l + 1]
                    src = (-l) % S  # slot of chunk 0 this layer

                    g = g_all[:, l : l + 1]
                    absrc = absrc_all[:, l : l + 1]
                    # dW = mx-mn; m* = mx - g*dW
                    dW = sbuf1("dW")
                    nc.vector.tensor_sub(dW[:], mx[:], mn[:])
                    mstar = sbuf1("mstar")
                    nc.vector.tensor_mul(mstar[:], g, dW[:])
                    nc.vector.tensor_sub(mstar[:], mx[:], mstar[:])
                    # Delta = dW*|rc|; s = 1/(Delta+eps); q = s*rc; A = Delta*s
                    Dl = sbuf1("Dl")
                    nc.vector.tensor_mul(Dl[:], dW[:], absrc)
                    De = sbuf1("De")
                    nc.vector.tensor_scalar_add(De[:], Dl[:], EPS)
                    s = sbuf1("s")
                    nc.vector.reciprocal(s[:], De[:])
                    q = sbuf1("q")
                    nc.vector.tensor_mul(q[:], s[:], rc[:])
                    A = sbuf1("A")
                    nc.vector.tensor_mul(A[:], Dl[:], s[:])
                    # b = -(q*m*); btl = b + dhat
                    bq = sbuf1("bq")
                    nc.vector.tensor_mul(bq[:], q[:], mstar[:])
                    nc.vector.tensor_scalar_mul(bq[:], bq[:], -1.0)
                    btl = sbuf1("btl")
                    nc.vector.tensor_add(btl[:], bq[:], dhat)
                    if not last:
                        # endpoint max for next layer: mx' = max(0, A*(A+dhat))
                        # (independent of the scan - compute it up front)
                        e = sbuf1("e")
                        nc.vector.tensor_scalar(e[:], A[:], dhat, None, Alu.add)
                        nc.vector.tensor_mul(e[:], e[:], A[:])
                        mx = sbuf1("mx")
                        nc.vector.tensor_scalar_max(mx[:], e[:], 0.0)

                    slots = st.tile([128, nchunk], f32, tag="slots", name="slots")
                    for k in range(nchunk):
                        rd = W[:, ((k + src) % S) * CW :][:, :CW]
                        wr = W[:, ((k + src - 1) % S) * CW :][:, :CW]
                        nc.vector._custom_dve(
                            fused,
                            out=wr,
                            in0=rd,
                            in1=bq[:],
                            s0=q[:],
                            s1=btl[:],
                            accum_out=slots[:, k : k + 1],
                        )

                    if not last:
                        # next-layer stats
                        rmn = sbuf1("rmn")
                        nc.vector.tensor_reduce(
                            rmn[:], slots[:], axis=AX.X, op=Alu.min
                        )
                        nc.vector.tensor_scalar_mul(rmn[:], rmn[:], -1.0)
                        nmn2 = sbuf1("nmn2")
                        nc.gpsimd.partition_all_reduce(
                            nmn2[:], rmn[:], 128, bass_isa.ReduceOp.max
                        )
                        mn = sbuf1("mn")
                        nc.vector.tensor_scalar_mul(mn[:], nmn2[:], -1.0)
                        rc = a2c
                    else:
                        # y = a2*W' + a0, chunked so the store overlaps
                        # (data is back at slots 0..nchunk-1)
                        for k in range(nchunk):
                            seg = W[:, k * CW : (k + 1) * CW]
                            nc.scalar.activation(
                                seg, seg, Act.Identity, bias=a0c, scale=a2c
                            )
                            nc.sync.dma_start(
                                out=ys[ch][:, k * CW : (k + 1) * CW], in_=seg
                            )

    nc.compile()
    return nc


_NC_CACHE = {}


def _get_nc():
    key = "full"
    if key not in _NC_CACHE:
        _NC_CACHE[key] = build_nc()
    return _NC_CACHE[key]


def shard_inputs(x, w0, w1, w2):
    """Full inputs -> list of per-core in_maps (channel sharding)."""
    x = np.ascontiguousarray(x, dtype=np.float32)
    in_maps = []
    for k in range(N_CORES):
        cols = slice(CH_PER_CORE * k, CH_PER_CORE * (k + 1))
        xk = np.ascontiguousarray(x[:, cols].transpose(1, 0, 2, 3)).reshape(
            CH_PER_CORE, 128, F_FULL
        )
        m = {"xs": xk}
        for nm, w in (("w0b", w0), ("w1b", w1), ("w2b", w2)):
            wc = np.asarray(w, dtype=np.float32)[:, cols]  # [20, 4]
            m[nm] = np.ascontiguousarray(
                np.broadcast_to(wc.T[:, None, :], (CH_PER_CORE, 128, N_LAYERS))
            )
        in_maps.append(m)
    return in_maps


def unshard_output(results):
    out = np.empty((B, C, H, Wd), dtype=np.float32)
    for k in range(N_CORES):
        ysk = np.asarray(results[k]["ys"], dtype=np.float32).reshape(
            CH_PER_CORE, B, H, Wd
        )
        out[:, CH_PER_CORE * k : CH_PER_CORE * (k + 1)] = ysk.transpose(1, 0, 2, 3)
    return out


def run_sharded(in_maps, trace=False, trace_kwargs=None):
    _import_concourse()
    from concourse.bass_utils import run_bass_kernel_spmd

    nc = _get_nc()
    return run_bass_kernel_spmd(
        nc,
        in_maps,
        core_ids=list(range(N_CORES)),
        trace=trace,
        **(trace_kwargs or {}),
    )


def kernel(x, w0, w1, w2):
    in_maps = shard_inputs(x, w0, w1, w2)
    res = run_sharded(in_maps)
    return unshard_output(res.results)
